# revision 1
# baseline (speedup 1.0000x reference)
"""Trainium2 Bass kernel for the colorization loss — v2.

Math (same restructure as v1, validated to rel-err ~1e-6):
  m(q)  = 2*a*gx_q + 2*b*gy_q - |g_q|^2        # = (a^2+b^2) - d^2(q)
  top-5 largest m == 5 nearest bins (sorted);  e_k = exp((m_k - m_0)/50)
  loss  = mean( (lse * sum_k(reb_k e_k) - sum_k(reb_k e_k zbar_k)) / sum_k e_k )
  with lse = log(sum_q exp(zbar_q)).

v2 layout (per core, 2 images = 32768 pixels):
  zbar shipped as bf16 (halves the dominant HBM stream), loaded in 16-tile
  groups on the SP ring.  ACT does ONE batched exp per group into a
  320-padded bf16 buffer; the per-tile sums run as a binary add-tree on the
  otherwise-idle Pool engine, which also runs nearly all the epilogue math.
  PE matmuls are fp32r (1 cycle/row; needs an even moving size, hence the
  314-col gamut block with a -1e30 pad bin).  abx uses 2 lhsT partition
  segments (bases 0/64), gamut block leading each segment so the matmuls
  carry a single DMA wait; seg0 arrives in 4 column pieces interleaved
  with the zbar stream, seg1 on the ACT/SWDGE rings mid-flight.  DVE does
  only the 256 top-8 selections plus reciprocal.  Leftover multi-wait
  instructions are legalized by bass_rust.generate_event_semaphores.
"""

import numpy as np
import ml_dtypes

import concourse.bass as bass
import concourse.tile as tile
from concourse import mybir
from concourse.bass_utils import run_bass_kernel_spmd

# Problem shape (hardcoded: nn_ColorizationLoss, B,H,W,Q = 16,128,128,313)
B, H, W, Q = 16, 128, 128, 313
NCORES = 8
B_PER = B // NCORES            # 2 images per core
PIX = B_PER * H * W            # 32768 pixels per core
P = 128                        # SBUF partitions / pixels per tile
NT = PIX // P                  # 256 tiles per core
GT = 16                        # tiles per zbar group
NG = NT // GT                  # 16 groups
TB = 32                        # tiles per epilogue batch
NB = NT // TB                  # 8 batches
TOPK = 5
INV50 = 1.0 / 50.0             # 1/(2*sigma^2), sigma=5
NSEG = 2                       # lhsT partition segments (bases 0/64)
SPIX = PIX // NSEG             # pixels per segment
QM = Q + 1                     # gamut block padded to an even 314 cols
                               # (fp32r matmul needs an even moving size);
                               # the pad bin has m = -1e30, never in top-8
SEGC = QM + SPIX               # segment row: [gamut | pixels]
QPAD = 320                     # es padded to 320 cols for a clean add-tree
ESBUF = 5                      # rotating es buffers

f32 = mybir.dt.float32
bf16 = mybir.dt.bfloat16
f32r = mybir.dt.float32r
AF = mybir.ActivationFunctionType
AX = mybir.AxisListType

_NC = None


def _build_nc():
    nc = bass.Bass()
    zbar_d = nc.dram_tensor("zbar", [PIX, Q], bf16, kind="ExternalInput")
    # 2 segments x rows [2gx; 2gy; -|g|^2] | [a; b; 1] per pixel half
    abx_d = nc.dram_tensor("abx", [3 * NSEG, SEGC], f32r, kind="ExternalInput")
    zf_d = nc.dram_tensor("zf", [P, NT * TOPK], f32, kind="ExternalInput")
    reb_d = nc.dram_tensor("rebt", [P, TB * TOPK], f32, kind="ExternalInput")
    out_d = nc.dram_tensor("acc", [P, NB + GT], f32, kind="ExternalOutput")

    # zbar viewed as [group g][partition p][tile-in-group j][q]
    zbar_g = zbar_d[:, :].rearrange("(g j p) q -> g p j q", j=GT, p=P)

    with tile.TileContext(nc) as tc:
        with (
            tc.tile_pool(name="singles", bufs=1) as singles,
            tc.tile_pool(name="zg", bufs=4) as zgp,
            tc.tile_pool(name="tree", bufs=2) as trp,
            tc.tile_pool(name="epi", bufs=2) as epi,
            tc.tile_pool(name="ps", bufs=3, space="PSUM") as psp,
            tc.tile_pool(name="psb", bufs=1, space="PSUM") as psbp,
        ):
            # abx: segment s occupies partitions 64s..64s+2; the gamut
            # block leads each segment so one DMA covers lhsT+rhs.
            abx_sb = singles.tile([3 + 64 * (NSEG - 1), SEGC], f32r)
            # seg0 in 4 pieces on SP, interleaved with the early zg loads:
            # A (gamut + tiles 0-7), A2 (tiles 8-31), B (32-63), C (64-127)
            CA0 = QM + P
            CAA = QM + 8 * P
            CA = QM + 32 * P
            CB = QM + 64 * P
            # first piece split across two rings in parallel: the tile-0
            # matmul only needs gamut+tile0, so the max8 stream starts
            # ~0.4us earlier; tiles 1-7 arrive via the idle ACT ring
            nc.sync.dma_start(out=abx_sb[0:3, 0:QM], in_=abx_d[0:3, 0:QM])
            nc.scalar.dma_start(out=abx_sb[0:3, QM:CA0], in_=abx_d[0:3, QM:CA0])
            nc.scalar.dma_start(out=abx_sb[0:3, CA0:CAA], in_=abx_d[0:3, CA0:CAA])

            zf_sb = singles.tile([P, NT, TOPK], f32)
            nc.gpsimd.dma_start(
                out=zf_sb, in_=zf_d[:, :].rearrange("p (t k) -> p t k", k=TOPK)
            )
            reb_sb = singles.tile([P, TB, TOPK], f32)
            nc.gpsimd.dma_start(
                out=reb_sb, in_=reb_d[:, :].rearrange("p (t k) -> p t k", k=TOPK)
            )
            acc = singles.tile([P, NB + GT], f32)

            # Full-size result buffers (never recycled => no WAR hazards)
            Sf = singles.tile([P, NT], f32)          # sum_q exp(zbar)
            Wf = singles.tile([P, NT, 8], f32)       # top-8 of m
            Xf = singles.tile([P, NT, TOPK], f32)    # m_k - m_0

            # Rotating es buffers; pad cols 313:320 zeroed once, and never
            # written again (exp writes only 0:313), so the add-tree can
            # fold a clean 320-wide block.
            es_bufs = []
            for i in range(ESBUF):
                e = singles.tile([P, GT, QPAD], bf16, name=f"es{i}")
                nc.gpsimd.memset(e[:, :, Q:QPAD], 0.0)
                es_bufs.append(e)

            zg_pending = {}

            def issue_zg(g):
                zgt = zgp.tile([P, GT, Q], bf16, tag="zg", name=f"zg{g}")
                nc.sync.dma_start(out=zgt, in_=zbar_g[g])
                zg_pending[g] = zgt

            CAB = QM + 16 * P
            nc.gpsimd.dma_start(out=abx_sb[0:3, CAA:CAB], in_=abx_d[0:3, CAA:CAB])
            nc.sync.dma_start(out=abx_sb[0:3, CAB:CA], in_=abx_d[0:3, CAB:CA])
            issue_zg(0)
            nc.sync.dma_start(out=abx_sb[0:3, CA:CB], in_=abx_d[0:3, CA:CB])
            issue_zg(1)

            H1 = QM + SPIX // 2
            for g in range(NG):
                if g + 2 < NG:
                    issue_zg(g + 2)
                zg = zg_pending.pop(g)
                seg = g // (NG // NSEG)
                so = 64 * seg

                # PE: 16 matmuls; fresh psum tag at segment boundaries keeps
                # the slot-recycle DVE wait off the abx-DMA-waiting matmul
                for j in range(GT):
                    t = g * GT + j
                    col = QM + (t % (NT // NSEG)) * P
                    # fresh psum slot for the first matmul after each abx
                    # piece: its DMA wait must be the instruction's ONLY sem
                    # wait (LDWEIGHTS encodes a single wait), so the slot
                    # recycle's DVE wait has to be structurally absent
                    if t in (0, 8, 32, 64, 128):
                        ps = psbp.tile([P, QM], f32, tag=f"psb{t}", name=f"psb{t}")
                    else:
                        ps = psp.tile([P, QM], f32, tag="ps")
                    nc.tensor.matmul(
                        ps,
                        abx_sb[so:so + 3, col:col + P],
                        abx_sb[so:so + 3, 0:QM],
                        start=True,
                        stop=True,
                    )
                    nc.vector.max(out=Wf[:, t, :], in_=ps[:, 0:Q])

                # ACT: one batched exp for the whole group
                es = es_bufs[g % ESBUF]
                nc.scalar.activation(out=es[:, :, 0:Q], in_=zg, func=AF.Exp)
                if g == 1:
                    # seg0 piece C (tiles 64-127) on the ACT ring, right
                    # after exp g1 (needed by the matmuls from tile 64 on)
                    nc.scalar.dma_start(out=abx_sb[0:3, CB:SEGC],
                                        in_=abx_d[0:3, CB:SEGC])

                # Pool: binary add-tree 320->160->...->5, then 5->1
                gsl = slice(g * GT, (g + 1) * GT)
                src = es
                w = QPAD // 2
                lvl = 0
                while w >= 5:
                    dst = trp.tile([P, GT, w], bf16, tag=f"tl{lvl}", name=f"tl{lvl}")
                    nc.gpsimd.tensor_add(dst, src[:, :, 0:w], src[:, :, w:2 * w])
                    src = dst
                    w //= 2
                    lvl += 1
                # src is [P, GT, 5]
                t7 = trp.tile([P, GT, 2], bf16, tag="t7", name="t7")
                nc.gpsimd.tensor_add(t7, src[:, :, 0:2], src[:, :, 2:4])
                t8 = trp.tile([P, GT, 1], bf16, tag="t8", name="t8")
                nc.gpsimd.tensor_add(t8, t7[:, :, 0:1], t7[:, :, 1:2])
                nc.gpsimd.tensor_add(
                    Sf[:, gsl].rearrange("p (t one) -> p t one", one=1),
                    t8, src[:, :, 4:5],
                )
                # abx segment 1 loads on the SWDGE ring between Pool trees
                # (needed by the group-8 matmuls)
                if g in (2, 4):
                    h0, h1 = (0, H1) if g == 2 else (H1, SEGC)
                    nc.gpsimd.dma_start(out=abx_sb[64:67, h0:h1],
                                        in_=abx_d[3:6, h0:h1])

                # ---- epilogue: batches 0-6 are 32 tiles; batch 7 runs
                # as two 16-tile halves so most of its chain overlaps the
                # final group's max8 stream ----
                def emit_epi(sl, col, wb, sfx, raw=False):
                    # final half: sub on DVE (same engine as the last max =>
                    # no cross-engine hop opening the tail chain)
                    sub_eng = nc.vector if raw else nc.gpsimd
                    sub_eng.tensor_sub(
                        Xf[:, sl], Wf[:, sl, 0:TOPK],
                        Wf[:, sl, 0:1].broadcast_to([P, wb, TOPK]),
                    )
                    E = epi.tile([P, wb, TOPK], f32, tag=f"E{sfx}", name=f"E{sfx}")
                    nc.scalar.activation(out=E, in_=Xf[:, sl], func=AF.Exp,
                                         scale=INV50)
                    lse = epi.tile([P, wb], f32, tag=f"lse{sfx}", name=f"lse{sfx}")
                    nc.scalar.activation(out=lse, in_=Sf[:, sl], func=AF.Ln)

                    def pool_sum5(nm, x):
                        y2 = epi.tile([P, wb, 2], f32, tag=f"{nm}2{sfx}", name=f"{nm}2{sfx}")
                        nc.gpsimd.tensor_add(y2, x[:, :, 0:2], x[:, :, 2:4])
                        y1 = epi.tile([P, wb, 1], f32, tag=f"{nm}1{sfx}", name=f"{nm}1{sfx}")
                        nc.gpsimd.tensor_add(y1, y2[:, :, 0:1], y2[:, :, 1:2])
                        y0 = epi.tile([P, wb], f32, tag=f"{nm}0{sfx}", name=f"{nm}0{sfx}")
                        nc.gpsimd.tensor_add(
                            y0.rearrange("p (t one) -> p t one", one=1),
                            y1, x[:, :, 4:5])
                        return y0

                    # sw first: its Ln/exp(-ln) recip runs on ACT while the
                    # Pool continues with U/UZ/s1/s2
                    sw = pool_sum5("sw", E)
                    r = epi.tile([P, wb], f32, tag=f"r{sfx}", name=f"r{sfx}")
                    if raw:
                        # DVE is idle after its stream: one recip op beats
                        # the two-op Ln/exp(-ln) ACT chain on the tail
                        nc.vector.reciprocal(r, sw)
                    else:
                        nlsw = epi.tile([P, wb], f32, tag=f"nlsw{sfx}", name=f"nlsw{sfx}")
                        nc.scalar.activation(out=nlsw, in_=sw, func=AF.Ln)
                        nc.scalar.activation(out=r, in_=nlsw, func=AF.Exp,
                                             scale=-1.0)
                    U = epi.tile([P, wb, TOPK], f32, tag=f"U{sfx}", name=f"U{sfx}")
                    nc.gpsimd.tensor_mul(U, E, reb_sb[:, 0:wb])
                    UZ = epi.tile([P, wb, TOPK], f32, tag=f"UZ{sfx}", name=f"UZ{sfx}")
                    nc.gpsimd.tensor_mul(UZ, U, zf_sb[:, sl])
                    s1 = pool_sum5("s1", UZ)
                    s2 = pool_sum5("s2", U)
                    t1 = epi.tile([P, wb], f32, tag=f"t1{sfx}", name=f"t1{sfx}")
                    nc.gpsimd.tensor_mul(t1, lse, s2)
                    t1b = epi.tile([P, wb], f32, tag=f"t1b{sfx}", name=f"t1b{sfx}")
                    nc.gpsimd.tensor_sub(t1b, t1, s1)
                    if raw:
                        # final half: per-tile values straight into output
                        # columns (host sums) — keeps the tail chain short
                        nc.gpsimd.tensor_mul(acc[:, col:col + wb], t1b, r)
                        return
                    nc.gpsimd.tensor_mul(t1b, t1b, r)
                    pp = t1b
                    w2 = wb // 2
                    lv = 0
                    while w2 >= 1:
                        nxt = epi.tile([P, w2], f32, tag=f"pp{lv}{sfx}", name=f"pp{lv}{sfx}")
                        nc.gpsimd.tensor_add(nxt, pp[:, 0:w2], pp[:, w2:2 * w2])
                        pp = nxt
                        w2 //= 2
                        lv += 1
                    nc.gpsimd.tensor_copy(acc[:, col:col + 1], pp)

                if g % 2 == 1 and g < NG - 1:
                    bi = g // 2
                    emit_epi(slice(bi * TB, (bi + 1) * TB), bi, TB, "")
                elif g == NG - 2:
                    emit_epi(slice(g * GT, (g + 1) * GT), NB - 1, GT, "h")
                elif g == NG - 1:
                    # 15 tiles' chains run under the final maxes; only the
                    # very last tile's width-1 chain trails the stream
                    emit_epi(slice(g * GT, g * GT + 15), NB, 15, "h", raw=True)
                    emit_epi(slice(g * GT + 15, g * GT + 16), NB + 15, 1, "z",
                             raw=True)

            nc.sync.dma_start(out=out_d[:, :], in_=acc)

    # Kernel-tail drain waits exceed the instruction sync-wait capacity; the
    # final out DMA is downstream of everything (acc is the sink), so its
    # completion sem alone suffices.  Find that DMA's sem from its on_update.
    out_sems = set()
    for blk in nc.m.functions[0].blocks:
        for inst in blk.instructions:
            si = getattr(inst, "sync_info", None)
            if si is None or type(inst).__name__ != "InstDMACopy":
                continue
            try:
                if inst.outs[0].memref == "acc":
                    out_sems |= {u.ant_name for u in si.on_update}
            except Exception:
                pass
    assert out_sems, "could not locate the output DMA's completion sem"
    for blk in nc.m.functions[0].blocks:
        for inst in blk.instructions:
            si = getattr(inst, "sync_info", None)
            if si is None or type(inst).__name__ != "InstDrain":
                continue
            ge = [w for w in si.on_wait if w.wait_mode == "sem-ge-imm"]
            if len(ge) >= 2:
                keep = [w for w in ge if w.ant_name in out_sems]
                assert keep, f"tail drain has no out-DMA wait: {ge}"
                si.on_wait = keep[:1]
    # TRN2 allows a single sem wait per instruction (2 on EventSemaphore);
    # split any excess waits onto preceding InstEventSemaphore instructions.
    import bass_rust
    bass_rust.generate_event_semaphores(nc)
    return nc


def _get_nc():
    global _NC
    if _NC is None:
        _NC = _build_nc()
    return _NC


def make_in_maps(Zbar, Y, rebalance, gamut):
    Zbar = np.asarray(Zbar, dtype=np.float32)
    Y = np.asarray(Y, dtype=np.float32)
    rebalance = np.asarray(rebalance, dtype=np.float32)
    gamut = np.asarray(gamut, dtype=np.float32)

    gx, gy = gamut[:, 0], gamut[:, 1]
    rhs = np.stack([2.0 * gx, 2.0 * gy, -(gx * gx + gy * gy)]).astype(np.float32)
    rebt = np.ascontiguousarray(
        np.broadcast_to(np.tile(rebalance[:TOPK], TB)[None, :], (P, TB * TOPK))
    ).astype(np.float32)

    in_maps = []
    for c in range(NCORES):
        sl = slice(c * B_PER, (c + 1) * B_PER)
        zb = Zbar[sl].reshape(PIX, Q)
        zb16 = np.ascontiguousarray(zb.astype(ml_dtypes.bfloat16))
        zf = np.ascontiguousarray(
            zb[:, 0:TOPK].reshape(NT, P, TOPK).transpose(1, 0, 2).reshape(P, NT * TOPK)
        )
        a = Y[sl, 1].reshape(PIX)
        b = Y[sl, 2].reshape(PIX)
        abx = np.zeros((3 * NSEG, SEGC), np.float32)
        for s in range(NSEG):
            px = slice(s * SPIX, (s + 1) * SPIX)
            abx[3 * s:3 * s + 3, 0:Q] = rhs
            abx[3 * s + 2, Q] = -1.0e30
            abx[3 * s + 0, QM:] = a[px]
            abx[3 * s + 1, QM:] = b[px]
            abx[3 * s + 2, QM:] = 1.0
        in_maps.append({"zbar": zb16, "abx": abx, "zf": zf, "rebt": rebt})
    return in_maps


def kernel(Zbar, Y, rebalance, gamut):
    in_maps = make_in_maps(Zbar, Y, rebalance, gamut)
    res = run_bass_kernel_spmd(_get_nc(), in_maps, list(range(NCORES)))
    total = sum(float(r["acc"].sum(dtype=np.float64)) for r in res.results)
    return np.float32(total / (B * H * W))

